# revision 2
# baseline (speedup 1.0000x reference)
"""PixelAttention Trainium2 kernel, v2.

One image per NeuronCore (data parallel over batch 8). Per core:
    seq  = image.reshape(C, T).T            # T = 1024, C = 256
    kqv  = seq @ w_kqv (+ bq on q only; bk cancels in softmax)
    causal 8-head attention, head_dim 32
    out  = mix(attn) + bm' + seq            # bm' = bm + bv @ w_mix (bv folds out)

Schedule: exp is the scarce resource (ACT-only exact exp would be ~31us),
so softmax exp is split across ACT (exact) and DVE/Pool (Schraudolph
bit-trick exp: i16 = trunc(A*L + B) bitcast to bf16, ~4% per-weight err,
~1e-3 end-to-end). PE work (QK 4-packed by head via tile_position row
quadrants, PV 2-packed by column) is interleaved with projections and the
mix tail to keep the tensor engine continuously busy (p-state ramp).
"""

import numpy as np
import ml_dtypes

import concourse.bass as bass
import concourse.tile as tile
from concourse import bacc, mybir
from concourse.bass_utils import run_bass_kernel_spmd

BF = ml_dtypes.bfloat16
T, C, H, D = 1024, 256, 8, 32
N_CORES = 8

# Schraudolph exp constants (bf16 bit pattern out of int16 affine)
A_S = 184.6649652337873
B_S = 16248.872096

# exp tile engine pattern: A=ACT exact, D=DVE schraudolph, P=Pool schraudolph
EXP_PAT = "ADADA"

_CACHE = {}


def _build_nc(exp_pat=EXP_PAT):
    f32 = mybir.dt.float32
    bf16 = mybir.dt.bfloat16
    i16 = mybir.dt.int16
    f8 = mybir.dt.float8e4
    f32r = mybir.dt.float32r
    DR = mybir.MatmulPerfMode.DoubleRow
    EXP = mybir.ActivationFunctionType.Exp
    MUL = mybir.AluOpType.mult
    ADD = mybir.AluOpType.add

    nc = bacc.Bacc("TRN2", target_bir_lowering=False, debug=False)

    def din(name, shape, dt):
        return nc.dram_tensor(name, shape, dt, kind="ExternalInput").ap()

    xb_d = din("xb", [128, 2 * T], bf16)        # [p, (a t)]
    wkq_d = din("wkq", [128, 1024], bf16)       # [p, (a w j)] w in {k,q}
    wvm_d = din("wvm", [128, 1024], bf16)       # [p, (a w j)] w in {v,m}
    bias4_d = din("bias4", [128, 4], f32)       # bq0 bq1 bm0 bm1
    tri_d = din("tri", [128, 128], bf16)        # tri[p, q] = q >= p
    id_d = din("id128", [128, 128], bf16)
    sel4_d = din("sel4", [4, 128], bf16)        # sel4[hl, m] = 32hl <= m < 32hl+32
    y = nc.dram_tensor("y", [128, 2 * T], f32, kind="ExternalOutput").ap()

    with tile.TileContext(nc) as tc:
        with (
            tc.tile_pool(name="consts", bufs=1) as consts,
            tc.tile_pool(name="sb", bufs=6) as sb,
            tc.tile_pool(name="lpp", bufs=3, space="PSUM") as lp_pool,
            tc.tile_pool(name="pop", bufs=2, space="PSUM") as po_pool,
        ):
            # ---- DMA loads: per-partition contiguous, split by partition halves ----
            xb = consts.tile([128, 2, T], bf16)
            wkq = consts.tile([128, 2, 2, 256], bf16)
            wvm = consts.tile([128, 2, 2, 256], bf16)
            bias4 = consts.tile([128, 4], f32)
            tri_sb = consts.tile([128, 128], bf16)
            sel4f = consts.tile([4, 128], bf16)

            id_sb = consts.tile([128, 128], bf16)

            wkq_r = wkq_d.rearrange("p (w a j) -> p w a j", w=2, a=2)
            xb_r = xb_d.rearrange("p (a t) -> p a t", a=2)
            wvm_r = wvm_d.rearrange("p (w a j) -> p w a j", w=2, a=2)
            # first-needed first: wk, xb t-half 0, wq, xb t-half 1, then the rest
            nc.sync.dma_start(out=wkq[0:64, 0], in_=wkq_r[0:64, 0])
            nc.scalar.dma_start(out=wkq[64:128, 0], in_=wkq_r[64:128, 0])
            nc.gpsimd.dma_start(out=xb[0:64, :, 0:512], in_=xb_r[0:64, :, 0:512])
            nc.sync.dma_start(out=xb[64:128, :, 0:512], in_=xb_r[64:128, :, 0:512])
            nc.scalar.dma_start(out=wkq[:, 1], in_=wkq_r[:, 1])
            nc.gpsimd.dma_start(out=xb[:, :, 512:1024], in_=xb_r[:, :, 512:1024])
            nc.scalar.dma_start(out=bias4, in_=bias4_d)
            nc.gpsimd.dma_start(out=wvm[:, 0], in_=wvm_r[:, 0])
            nc.sync.dma_start(out=tri_sb, in_=tri_d)
            nc.scalar.dma_start(out=sel4f, in_=sel4_d)
            nc.gpsimd.dma_start(out=wvm[:, 1], in_=wvm_r[:, 1])
            nc.sync.dma_start(out=id_sb, in_=id_d)

            # warm the exp table before logits arrive
            warm = consts.tile([128, 1], f32)
            nc.vector.memset(warm, 0.0)
            nc.scalar.activation(out=warm, in_=warm, func=EXP)

            qkT = consts.tile([128, 4, T], bf16)   # 0-1: kT jl, 2-3: qT jl
            vsb = consts.tile([128, 8, H, 33], bf16)
            nc.vector.memset(vsb[:, :, :, 32:33], 1.0)
            attnT = consts.tile([128, 2, T], bf16)

            tri_b = bass.AP(
                tensor=tri_sb.tensor, offset=tri_sb.offset,
                ap=[list(tri_sb.ap[0]), [0, 2]] + list(tri_sb.ap[1:]),
            )

            def eng_copy(eng, out, in_):
                if eng is nc.scalar:
                    eng.copy(out=out, in_=in_)
                else:
                    eng.tensor_copy(out=out, in_=in_)

            # ---- aux op emitters ----
            # proj group dst: 0 kT jl0, 1 kT jl1, 2 qT jl0, 3 qT jl1
            def kq_group(dst, tch, copy_eng):
                w_idx = dst // 2          # 0 -> k, 1 -> q
                jl = dst % 2
                js = slice(jl * 128, (jl + 1) * 128)
                ts_ = slice(tch * 512, (tch + 1) * 512)
                p = lp_pool.tile([128, 2, 512], f32, tag="lp", name="pp")[:, 0, :]
                nc.tensor.matmul(
                    out=p, lhsT=wkq[:, w_idx, 0, js], rhs=xb[:, 0, ts_],
                    start=True, stop=False,
                )
                nc.tensor.matmul(
                    out=p, lhsT=wkq[:, w_idx, 1, js], rhs=xb[:, 1, ts_],
                    start=False, stop=True,
                )
                if w_idx == 1:  # q: add bias
                    copy_eng.tensor_scalar(
                        out=qkT[:, dst, ts_], in0=p,
                        scalar1=bias4[:, jl:jl + 1], scalar2=None, op0=ADD,
                    )
                else:
                    eng_copy(copy_eng, qkT[:, dst, ts_], p)

            def v_group(st, copy_eng):
                ss = slice(st * 128, (st + 1) * 128)
                p = lp_pool.tile([128, 2, 512], f32, tag="lp", name="pv")[:, 0, 0:256]
                nc.tensor.matmul(
                    out=p, lhsT=xb[:, 0, ss], rhs=wvm[:, 0, 0, :],
                    start=True, stop=False,
                )
                nc.tensor.matmul(
                    out=p, lhsT=xb[:, 1, ss], rhs=wvm[:, 0, 1, :],
                    start=False, stop=True,
                )
                eng_copy(copy_eng, vsb[:, st, :, 0:32],
                         p.rearrange("p (h e) -> p h e", e=32))


            # ---- phase 1 head: what's needed for (c0, g0) ----
            kq_order0 = [(0, 0), (0, 1), (2, 0), (2, 1)]
            kq_order1 = [(1, 0), (1, 1), (3, 0), (3, 1)]
            copy_cyc = [nc.scalar, nc.vector, nc.scalar, nc.vector]
            for i, (dst, tch) in enumerate(kq_order0):
                kq_group(dst, tch, nc.vector if dst >= 2 else copy_cyc[i % 2])
            for st in range(4):
                v_group(st, copy_cyc[st % 2])

            aux = [("kq", 1, 0), ("kq", 1, 1), ("kq", 3, 0), ("kq", 3, 1)] + [
                ("v", st, None) for st in range(4, 8)
            ]
            aux_i = [0]

            def emit_aux(n=1):
                for _ in range(n):
                    if aux_i[0] < len(aux):
                        kind, a0, a1 = aux[aux_i[0]]
                        aux_i[0] += 1
                        if kind == "kq":
                            kq_group(a0, a1, nc.vector if a0 >= 2 else nc.scalar)
                        else:
                            v_group(a0, copy_cyc[a0 % 2])

            # ---- attention ----
            exp_i = [0]
            mask_i = [0]

            def emit_exp(E, lp, tlo, diag):
                if diag:
                    use_dve = mask_i[0] % 2 == 0
                    mask_i[0] += 1
                    if use_dve:
                        nc.vector.tensor_scalar(
                            out=E[:, :, tlo:512].bitcast(i16),
                            in0=lp[:, :, tlo:512],
                            scalar1=A_S, scalar2=B_S, op0=MUL, op1=ADD,
                        )
                        nc.vector.tensor_mul(
                            out=E[:, :, tlo:tlo + 128],
                            in0=E[:, :, tlo:tlo + 128], in1=tri_b,
                        )
                    else:
                        nc.scalar.activation(
                            out=E[:, :, tlo:512], in_=lp[:, :, tlo:512],
                            func=EXP,
                        )
                        nc.gpsimd.tensor_mul(
                            out=E[:, :, tlo:tlo + 128],
                            in0=E[:, :, tlo:tlo + 128], in1=tri_b,
                        )
                    return
                pat = exp_pat[exp_i[0] % len(exp_pat)]
                exp_i[0] += 1
                if pat == "A":
                    nc.scalar.activation(
                        out=E[:, :, tlo:512], in_=lp[:, :, tlo:512], func=EXP,
                    )
                else:
                    eng = nc.vector if pat == "D" else nc.gpsimd
                    eng.tensor_scalar(
                        out=E[:, :, tlo:512].bitcast(i16),
                        in0=lp[:, :, tlo:512],
                        scalar1=A_S, scalar2=B_S, op0=MUL, op1=ADD,
                    )

            dma_cyc = [nc.sync, nc.scalar]
            dma_i = [0]

            def next_dma():
                e = dma_cyc[dma_i[0] % len(dma_cyc)]
                dma_i[0] += 1
                return e

            ou_eng = [nc.scalar, nc.vector]
            pend_div = []
            pend_mix = []

            def emit_mix(c):
                cs = slice(c * 512, (c + 1) * 512)
                for c2t in range(2):
                    c2s = slice(c2t * 128, (c2t + 1) * 128)
                    mp = lp_pool.tile([128, 2, 512], f32, tag="lp",
                                      name="mp")[:, 0, :]
                    nc.tensor.matmul(
                        out=mp, lhsT=wvm[:, 1, 0, c2s], rhs=attnT[:, 0, cs],
                        start=True, stop=False,
                    )
                    if c2t == 0:
                        nc.tensor.matmul(
                            out=mp, lhsT=id_sb, rhs=xb[:, c2t, cs],
                            start=False, stop=False,
                        )
                    nc.tensor.matmul(
                        out=mp, lhsT=wvm[:, 1, 1, c2s], rhs=attnT[:, 1, cs],
                        start=False, stop=True,
                    )
                    os_ = sb.tile([128, 512], f32, tag="os", name="os", bufs=4)
                    if c2t == 0:
                        nc.scalar.activation(
                            out=os_, in_=mp,
                            func=mybir.ActivationFunctionType.Identity,
                            bias=bias4[:, 2:3], scale=1.0,
                        )
                    else:
                        nc.vector.scalar_tensor_tensor(
                            out=os_, in0=mp, scalar=bias4[:, 2 + c2t:3 + c2t],
                            in1=xb[:, c2t, cs], op0=ADD, op1=ADD,
                        )
                    next_dma().dma_start(
                        out=y.rearrange("p (a t) -> p a t", a=2)[:, c2t, cs],
                        in_=os_,
                    )

            # deferred division: the non-PE chain (drain, den extract, recip)
            # flushes early in the NEXT group; the PE op (bc) and the rest
            # flush later, once the chain has had time to complete, so the
            # in-order PE queue is never head-of-line blocked on it.
            pend_pre = [None]
            pend_pe = [None]

            def flush(slot):
                if slot[0] is not None:
                    fn = slot[0]
                    slot[0] = None
                    fn()

            for c in range(2):
                for g in range(2):
                    n_st = 4 * c + 4
                    po = {
                        0: po_pool.tile([128, 512], f32, tag="po", name="po0"),
                        1: po_pool.tile([128, 512], f32, tag="po", name="po1"),
                    }
                    stash = [None] * n_st
                    for st in range(n_st + 1):
                        if st == 1:
                            flush(pend_pre)
                        if st == 3:
                            flush(pend_pe)
                        if st < n_st:
                            tlo = max(0, 128 * st - 512 * c)
                            diag = (128 * st - 512 * c) >= 0
                            Es = []
                            for pair in range(2):
                                lp = lp_pool.tile(
                                    [128, 2, 512], f32, tag="lp", name="lp")
                                E = sb.tile(
                                    [128, 2, 512], bf16, tag="E", name="E", bufs=8)
                                for h2 in range(2):
                                    rp = 32 * (2 * pair + h2)
                                    nc.tensor.matmul(
                                        out=lp[:, h2, tlo:512],
                                        lhsT=qkT[rp:rp + 32, g,
                                                 st * 128:(st + 1) * 128],
                                        rhs=qkT[rp:rp + 32, 2 + g,
                                                c * 512 + tlo:(c + 1) * 512],
                                        start=True, stop=True,
                                        tile_position=(rp, 0),
                                    )
                                Es.append((E, lp, tlo, diag))
                            if c == 0:
                                emit_aux(2 if g == 0 else 1)
                            for E, lp, tlo_, diag_ in Es:
                                emit_exp(E, lp, tlo_, diag_)
                            stash[st] = Es
                        if st > 0:
                            pst = st - 1
                            ptlo = max(0, 128 * pst - 512 * c)
                            for pair in range(2):
                                E = stash[pst][pair][0]
                                for h2 in range(2):
                                    hg = 4 * g + 2 * pair + h2
                                    nc.tensor.matmul(
                                        out=po[pair][64 * h2:64 * h2 + 33,
                                                     ptlo:512],
                                        lhsT=vsb[:, pst, hg, :],
                                        rhs=E[:, h2, ptlo:512],
                                        start=(pst == 0), stop=(pst == n_st - 1),
                                        skip_group_check=True,
                                        tile_position=(0, 64 * h2),
                                    )
                            stash[pst] = None

                    # ---- division for (c, g), deferred into next group ----
                    def make_pre(c, g, po):
                        def pre():
                            ou = {}
                            for pair in range(2):
                                t_ou = sb.tile([128, 512], f32, tag="ou",
                                               name="ou", bufs=4)
                                eng_copy(ou_eng[pair], t_ou, po[pair])
                                ou[pair] = t_ou
                            ouc = sb.tile([128, 512], f32, tag="ouc",
                                          name="ouc", bufs=2)
                            rs4 = sb.tile([4, 512], f32, tag="rs", name="rs",
                                          bufs=2)
                            for pair in range(2):
                                for h2 in range(2):
                                    hl = 2 * pair + h2
                                    (nc.sync if hl % 2 else nc.gpsimd).dma_start(
                                        out=rs4[hl:hl + 1, :],
                                        in_=ou[pair][64 * h2 + 32:
                                                     64 * h2 + 33, :],
                                    )
                            for pair in range(2):
                                for h2 in range(2):
                                    hl = 2 * pair + h2
                                    (nc.sync if hl % 2 else nc.gpsimd).dma_start(
                                        out=ouc[32 * hl:32 * hl + 32, :],
                                        in_=ou[pair][64 * h2:64 * h2 + 32, :],
                                    )
                            rcf = sb.tile([4, 512], f32, tag="rcf",
                                          name="rcf", bufs=2)
                            nc.vector.reciprocal_approx_fast(out=rcf, in_=rs4)
                            rcb = sb.tile([4, 512], bf16, tag="rcb",
                                          name="rcb", bufs=2)
                            nc.scalar.copy(out=rcb, in_=rcf)
                            return ouc, rcb
                        return pre

                    def make_pe(c, g, pre_result):
                        def pe():
                            ouc, rcb = pre_result()
                            bc = lp_pool.tile([128, 2, 512], f32, tag="lp",
                                              name="bc")[:, 0, :]
                            nc.tensor.matmul(
                                out=bc, lhsT=sel4f, rhs=rcb,
                                start=True, stop=True,
                            )
                            cs = slice(c * 512, (c + 1) * 512)
                            nc.vector.tensor_mul(
                                out=attnT[:, g, cs], in0=ouc, in1=bc)
                            if g == 1:
                                emit_mix(c)
                        return pe

                    res_box = []

                    def make_pre_boxed(c, g, po, box):
                        base = make_pre(c, g, po)

                        def pre():
                            box.append(base())
                        return pre

                    pend_pre[0] = make_pre_boxed(c, g, po, res_box)
                    pend_pe[0] = make_pe(c, g, lambda b=res_box: b[0])

            flush(pend_pre)
            flush(pend_pe)

    nc.compile()
    return nc


def _host_inputs(image, w_kqv, b_kqv, w_mix, b_mix):
    s = np.float32(1.0 / np.sqrt(D))
    wk = np.ascontiguousarray(w_kqv[:, :256])
    wq = np.ascontiguousarray(w_kqv[:, 256:512]) * s
    wv = np.ascontiguousarray(w_kqv[:, 512:])
    wm = np.asarray(w_mix, np.float32)
    bq = (b_kqv[256:512] * s).astype(np.float32)
    bv = b_kqv[512:].astype(np.float32)
    bm = (np.asarray(b_mix, np.float32) + bv @ wm).astype(np.float32)

    # wkq[p, w, a, j] = W_w[a*128+p, j]  (w-major: k block then q block per partition)
    wkq = np.stack([wk, wq], axis=1).reshape(2, 128, 2, 256).transpose(1, 2, 0, 3)
    wvm = np.stack([wv, wm], axis=1).reshape(2, 128, 2, 256).transpose(1, 2, 0, 3)
    bias4 = np.stack(
        [bq[0:128], bq[128:256], bm[0:128], bm[128:256]], axis=1
    ).astype(np.float32)
    tri = (np.arange(128)[None, :] >= np.arange(128)[:, None])
    id128 = np.eye(128, dtype=np.float32)
    sel4 = np.zeros((4, 128), np.float32)
    for hl in range(4):
        sel4[hl, 32 * hl:32 * hl + 32] = 1.0
    common = {
        "wkq": np.ascontiguousarray(wkq.reshape(128, 1024)).astype(BF),
        "wvm": np.ascontiguousarray(wvm.reshape(128, 1024)).astype(BF),
        "bias4": np.ascontiguousarray(bias4),
        "tri": tri.astype(BF),
        "id128": id128.astype(BF),
        "sel4": sel4.astype(BF),
    }
    in_maps = []
    for i in range(N_CORES):
        x2 = np.asarray(image[i], np.float32).reshape(2, 128, T).transpose(1, 0, 2)
        in_maps.append(
            {**common, "xb": np.ascontiguousarray(x2.reshape(128, 2 * T)).astype(BF)}
        )
    return in_maps


def _run(inputs, trace=False):
    if "nc" not in _CACHE:
        _CACHE["nc"] = _build_nc()
    nc = _CACHE["nc"]
    in_maps = _host_inputs(
        np.asarray(inputs["image"], np.float32),
        np.asarray(inputs["w_kqv"], np.float32),
        np.asarray(inputs["b_kqv"], np.float32),
        np.asarray(inputs["w_mix"], np.float32),
        np.asarray(inputs["b_mix"], np.float32),
    )
    res = run_bass_kernel_spmd(nc, in_maps, list(range(N_CORES)), trace=trace)
    out = np.stack(
        [
            np.asarray(res.results[i]["y"])
            .reshape(128, 2, T).transpose(1, 0, 2).reshape(C, 32, 32)
            for i in range(N_CORES)
        ]
    ).astype(np.float32)
    return out, res


def kernel(**inputs):
    out, _ = _run(inputs, trace=False)
    return out
